# revision 68
# baseline (speedup 1.0000x reference)
"""Multi-head self-attention Trainium2 kernel (8 NeuronCores, tensor-parallel over heads).

Problem: x[2,2048,1024], W_qkv[3072,1024], b_qkv[3072], W_out[1024,1024], b_out[1024]
  qkv = x @ W_qkv.T + b_qkv ; per-head attention (16 heads, hd=64) ; out = ctx @ W_out.T + b_out

Sharding: head-parallel. Core c owns heads (2c, 2c+1) for both batches.
Each core computes its 2 heads' Q,K,V (full sequence), attention, and a partial
output projection (columns of W_out for its heads). Host sums the 8 partials
and adds b_out.

Numerics: bf16 inputs/weights/V/E/ctx; Q,K quantized to fp8e4m3 so the scores
matmul can use DoubleRow perf mode (half PE cost); fp32 psum accumulation and
fp32 output partials. Softmax denominators come free as a ones-column in the
AV matmul; reciprocal on DVE, partition-broadcast on GPSIMD.

Schedule: the ScalarE exp stream (128 x ~1.04us) is the pacing engine, so the
kernel starts it as early as possible: phase 1 computes only the batch-0 K,Q
projections for the first x-group pair and shuffles them into the DoubleRow
slab layout; the first attention chunk then starts while everything else
(V projections + transposes, batch-1 qkv, output projections, chunk tails)
drains through a unified filler stream interleaved between scores/AV matmuls.
Each chunk's last AV matmuls + softmax-scale chain run as fillers of the NEXT
chunk so the PE never waits on the scale chain at chunk boundaries.
"""
import sys
sys.path.insert(0, '/opt/trn_rl_repo')

import numpy as np
from contextlib import ExitStack
from collections import deque

import concourse.bass as bass
import concourse.bacc as bacc
import concourse.tile as tile
from concourse import mybir
from concourse.bass_utils import run_bass_kernel_spmd

import ml_dtypes

F32 = mybir.dt.float32
BF16 = mybir.dt.bfloat16
F8 = mybir.dt.float8e4
EXP = mybir.ActivationFunctionType.Exp
DR = mybir.MatmulPerfMode.DoubleRow
NP_BF = ml_dtypes.bfloat16

B, N, D = 2, 2048, 1024
BN = B * N            # 4096
HEADS, HD = 16, 64
NCORES = 8
HPC = HEADS // NCORES  # heads per core = 2
EPC = 3 * HPC * HD     # qkv rows per core = 384
SCALE = 1.0 / np.sqrt(HD)
LAG = 4                # AV trails scores by LAG kb-steps

_cached = {}


def build_nc():
    nc = bacc.Bacc("TRN2", target_bir_lowering=False, debug=False, num_devices=NCORES)
    xT = nc.declare_dram_parameter("xT", [D, BN], BF16, isOutput=False)
    wqkvT = nc.declare_dram_parameter("wqkvT", [D, EPC], BF16, isOutput=False)
    bqkv = nc.declare_dram_parameter("bqkv", [EPC, 1], F32, isOutput=False)
    woT = nc.declare_dram_parameter("woT", [HPC * HD, D], BF16, isOutput=False)
    ident = nc.declare_dram_parameter("ident", [128, 128], BF16, isOutput=False)
    out = nc.declare_dram_parameter("out", [BN, D], F32, isOutput=True)

    with tile.TileContext(nc) as tc, ExitStack() as ctx:
        singles = ctx.enter_context(tc.tile_pool(name="singles", bufs=1))

        wq_sb = singles.tile([128, 8, EPC], BF16)      # [d-part, d-tile, e]
        bq_sb = singles.tile([128, 3], F32)
        id_sb = singles.tile([128, 128], BF16)
        woT_sb = singles.tile([128, D], BF16)
        xg = singles.tile([128, 8, BN], BF16)

        def load_x(g, d0, d1, eng=None):
            (eng or nc.sync).dma_start(
                out=xg[:, d0:d1, g * 1024:(g + 1) * 1024],
                in_=xT[d0 * 128:d1 * 128, g * 1024:(g + 1) * 1024].rearrange(
                    "(t p) c -> p t c", p=128))

        def load_wq(d0, d1, e0, e1):
            nc.sync.dma_start(
                out=wq_sb[:, d0:d1, e0:e1],
                in_=wqkvT[d0 * 128:d1 * 128, e0:e1].rearrange(
                    "(t p) e -> p t e", p=128))

        # load order = DMA service order: everything the first chunk needs
        # first (K,Q weight rows only; V rows after batch-0 x), then x groups.
        load_wq(0, 2, 0, 256)
        load_x(0, 0, 2)
        load_wq(2, 8, 0, 256)
        load_x(0, 2, 4)
        nc.sync.dma_start(out=bq_sb, in_=bqkv[:, :].rearrange("(t p) o -> p (t o)", p=128))
        load_x(0, 4, 8)
        nc.sync.dma_start(out=id_sb, in_=ident[:, :])
        load_wq(0, 8, 256, 384)
        load_x(1, 0, 8)

        QT8 = singles.tile([128, BN], F8)
        KT8 = singles.tile([128, BN], F8)
        VT = singles.tile([128, BN], BF16)
        Q2 = singles.tile([64, 2, BN], F8)   # DoubleRow slabs [32*head, slab, n]
        K2 = singles.tile([64, 2, BN], F8)
        V2 = singles.tile([128, 32, 130], BF16)
        nc.gpsimd.memset(V2[:, :, 64:65], 1.0)
        nc.gpsimd.memset(V2[:, :, 129:130], 1.0)

        def qkv_mm(ps, m, h, g, d0, d1):
            for d in range(d0, d1):
                nc.tensor.matmul(
                    ps,
                    wq_sb[:, d, m * 128:(m + 1) * 128],
                    xg[:, d, g * 1024 + h * 512: g * 1024 + (h + 1) * 512],
                    start=(d == 0), stop=(d == 7))

        IDENT = mybir.ActivationFunctionType.Identity

        def evac(ps, m, h, g, scalar=False):
            tgt = [QT8, KT8, VT][m]
            cols = bass.ds(g * 1024 + h * 512, 512)
            if scalar:
                # ScalarE evacuation (phase 1 only, while exp is idle)
                nc.scalar.activation(tgt[:, cols], ps, IDENT,
                                     bias=bq_sb[:, m:m + 1])
            else:
                nc.vector.tensor_scalar_add(tgt[:, cols], ps, bq_sb[:, m:m + 1])

        def shuffle_moves(m, g, eng=None):
            """One x-group's Q or K columns -> DoubleRow slab layout."""
            src, dst = ((QT8, Q2), (KT8, K2))[m == 1]
            cols = bass.ds(g * 1024, 1024)
            for h in range(2):
                for s in range(2):
                    (eng or nc.sync).dma_start(
                        out=dst[32 * h:32 * h + 32, s, cols],
                        in_=src[64 * h + 32 * s: 64 * h + 32 * s + 32, cols])

        # ---- phase 1: batch-0 K,Q for x-group 0 only (d-interleaved so the
        # matmuls chase the x loads), evac + shuffle per tensor; everything
        # else rides the chunk filler stream ----
        warm = singles.tile([128, 512], BF16)
        nc.gpsimd.memset(warm[:, 0:128], 0.0)
        with tc.tile_pool(name="psq", bufs=1, space="PSUM") as psq:
            # dummy matmuls bridge the PE p-state ramp until the x loads land
            pw = psq.tile([128, 512], F32, tag="warm", name="pw")
            for i in range(20):
                nc.tensor.matmul(pw, warm[:, 0:128], warm,
                                 start=(i == 0), stop=(i == 19))
            ps = {(m, h): psq.tile([128, 512], F32, tag=f"t{m}{h}", name="ps")
                  for m in (1, 0) for h in range(2)}
            # d 0-5 interleaved across groups (chasing the x loads), then
            # finish each group and evacuate immediately, K first so its
            # shuffle DMAs start while Q is still multiplying
            for d in range(6):
                for m in (1, 0):
                    for h in range(2):
                        qkv_mm(ps[(m, h)], m, h, 0, d, d + 1)
            for m in (1, 0):
                for h in range(2):
                    qkv_mm(ps[(m, h)], m, h, 0, 6, 8)
                    evac(ps[(m, h)], m, h, 0, scalar=True)
                shuffle_moves(m, 0)
            # V for x-group 0 while the shuffles land: PE and ScalarE are
            # otherwise idle here; V2 copies ride ScalarE too
            for h in range(2):
                pv = psq.tile([128, 512], F32, tag=f"t1{h}", name="pv")
                qkv_mm(pv, 2, h, 0, 0, 8)
                evac(pv, 2, h, 0, scalar=True)
                for kb in range(4 * h, 4 * h + 4):
                    pt = psq.tile([128, 128], BF16, tag="pt", name="pt")
                    nc.tensor.transpose(
                        pt, VT[:, kb * 128:(kb + 1) * 128], id_sb)
                    nc.scalar.activation(V2[:, kb, 0:64], pt[:, 0:64], IDENT)
                    nc.scalar.activation(V2[:, kb, 65:129], pt[:, 64:128],
                                         IDENT)



        # ---- chunk era ----
        with tc.tile_pool(name="pss", bufs=2, space="PSUM") as pss, \
             tc.tile_pool(name="psav", bufs=1, space="PSUM") as psav, \
             tc.tile_pool(name="fill", bufs=1, space="PSUM") as fill, \
             tc.tile_pool(name="epool", bufs=8) as epool, \
             tc.tile_pool(name="npool", bufs=3) as npool, \
             tc.tile_pool(name="cpool", bufs=6) as cpool, \
             tc.tile_pool(name="opool", bufs=3) as opool:

            def vtrans(kb):
                pt = fill.tile([128, 1024], BF16, tag="proj", name="pt")
                nc.tensor.transpose(pt[:, 0:128], VT[:, kb * 128:(kb + 1) * 128],
                                    id_sb)
                nc.vector.tensor_copy(V2[:, kb, 0:64], pt[:, 0:64])
                nc.vector.tensor_copy(V2[:, kb, 65:129], pt[:, 64:128])

            def qkv_group(m, h, g, then=()):
                """Filler callables for one (m,h,g) projection group."""
                st = {}
                cbs = []
                def alloc():
                    st["ps"] = fill.tile([128, 512], F32, tag="qkv", name="mq")
                cbs.append(alloc)
                for j in range(4):
                    cbs.append(lambda j=j, m=m, h=h, g=g:
                               qkv_mm(st["ps"], m, h, g, 2 * j, 2 * j + 2))
                cbs.append(lambda m=m, h=h, g=g: evac(st["ps"], m, h, g))
                cbs.extend(then)
                return cbs

            def proj_fillers(box, pb, pqb):
                """8 half-blocks of one chunk's output projection."""
                cbs = []
                for j in range(4):
                    st = {}
                    for eh in range(2):
                        def half(st=st, box=box, pb=pb, pqb=pqb, j=j, eh=eh):
                            po = fill.tile([128, 512], F32, tag="proj",
                                           name="po")
                            nc.tensor.matmul(
                                po, box["ctx"][:, j * 128:(j + 1) * 128],
                                woT_sb[:, eh * 512:(eh + 1) * 512],
                                start=True, stop=True)
                            if eh == 0:
                                st["ob"] = opool.tile([128, 1024], F32,
                                                      name="ob")
                            nc.vector.tensor_copy(
                                st["ob"][:, eh * 512:(eh + 1) * 512], po)
                            if eh == 1:
                                nb = pqb * 4 + j
                                nc.sync.dma_start(
                                    out=out[pb * N + nb * 128:
                                            pb * N + (nb + 1) * 128, :],
                                    in_=st["ob"])
                        cbs.append(half)
                return cbs

            def emit_chunk(b, qb, fillers, fine=False):
                """Scores/exp/AV for one 512-query chunk. Returns (box, tail):
                tail = last LAG AV pairs + scale chain, to run as the next
                chunk's leading fillers. fine=True (last chunk) skips the
                ch1 partition-move DMA; the final projection reads the two
                head halves as separate stationaries."""
                qs = bass.ds(b * N + qb * 512, 512)
                pav = [psav.tile([65, 512], F32, tag=f"pav{h}", name=f"pav{h}")
                       for h in range(2)]
                Elist = {}

                def drain(kb):
                    if fillers:
                        take = -(-len(fillers) // (4 * (17 - kb)))
                        for _ in range(min(take, len(fillers))):
                            fillers.popleft()()

                def av(kb):
                    kb32 = b * 16 + kb
                    Ep = Elist.pop(kb)
                    nc.tensor.matmul(pav[0], V2[:, kb32, 0:65], Ep[:, 0:512],
                                     start=(kb == 0), stop=(kb == 15))
                    nc.tensor.matmul(pav[1], V2[:, kb32, 65:130],
                                     Ep[:, 512:1024],
                                     start=(kb == 0), stop=(kb == 15))

                for kb in range(16):
                    ks = bass.ds(b * N + kb * 128, 128)
                    pS = pss.tile([128, 1024], F32, name="pS")
                    for h in range(2):
                        nc.tensor.matmul(
                            pS[:, h * 512:(h + 1) * 512],
                            K2[32 * h:32 * h + 32, :, ks],
                            Q2[32 * h:32 * h + 32, :, qs],
                            start=True, stop=True, perf_mode=DR)
                    E = epool.tile([128, 1024], BF16, name="E")
                    nc.scalar.activation(E, pS, EXP, scale=float(SCALE))
                    Elist[kb] = E
                    drain(kb)
                    if kb >= LAG:
                        av(kb - LAG)
                    drain(kb)

                box = {}
                tail = deque()
                for kb in range(16 - LAG, 16):
                    tail.append(lambda kb=kb: av(kb))

                def scale_chain():
                    rec = [npool.tile([1, 512], F32, tag=f"rec{h}",
                                      name=f"rec{h}") for h in range(2)]
                    Rb = [npool.tile([64, 512], F32, tag=f"rb{h}",
                                     name=f"rb{h}") for h in range(2)]
                    for h in range(2):
                        nc.vector.reciprocal(rec[h], pav[h][64:65, :])
                        nc.gpsimd.partition_broadcast(Rb[h], rec[h][0:1, :])
                    ctxT = cpool.tile([128, 512], BF16, tag="ctx", name="ctx")
                    ch1 = cpool.tile([64, 512], BF16, tag="ch1", name="ch1")
                    nc.vector.tensor_mul(ctxT[0:64, :], pav[0][0:64, :], Rb[0])
                    nc.vector.tensor_mul(ch1, pav[1][0:64, :], Rb[1])
                    if fine:
                        box["ch1"] = ch1
                    else:
                        nc.sync.dma_start(out=ctxT[64:128, :], in_=ch1)
                    box["ctx"] = ctxT
                tail.append(scale_chain)
                return box, tail

            # background work assigned per chunk index; ordering within
            # bg[0] is timing-critical: V-g0h0 first (vtrans 0-3 gate the
            # first AV), then K-g1 (gates scores kb8+), then the rest.
            # Early chunks are PE-heavy, so the early chunks' output
            # projections are deferred to the Act-bound late chunks.
            bg = {i: [] for i in range(8)}
            bg[0] += qkv_group(1, 0, 1) + qkv_group(
                1, 1, 1, [lambda: shuffle_moves(1, 1)])
            bg[0] += qkv_group(2, 0, 1, [lambda kb=kb: vtrans(kb)
                                         for kb in range(8, 12)])
            bg[0] += qkv_group(2, 1, 1, [lambda kb=kb: vtrans(kb)
                                         for kb in range(12, 16)])
            bg[1] += qkv_group(0, 0, 1) + qkv_group(
                0, 1, 1, [lambda: shuffle_moves(0, 1)])
            bg[1] += qkv_group(1, 0, 2) + qkv_group(1, 1, 2)
            bg[2] += qkv_group(0, 0, 2) + qkv_group(0, 1, 2)
            bg[2] += qkv_group(2, 0, 2, [lambda kb=kb: vtrans(kb)
                                         for kb in range(16, 20)])
            bg[2] += qkv_group(2, 1, 2, [lambda kb=kb: vtrans(kb)
                                         for kb in range(20, 24)])
            bg[2] += qkv_group(1, 0, 3) + qkv_group(
                1, 1, 3, [lambda: shuffle_moves(1, 2),
                          lambda: shuffle_moves(1, 3)])
            bg[3] += qkv_group(0, 0, 3) + qkv_group(
                0, 1, 3, [lambda: shuffle_moves(0, 2)])
            bg[3] += qkv_group(2, 0, 3, [lambda kb=kb: vtrans(kb)
                                         for kb in range(24, 28)])
            bg[3] += qkv_group(2, 1, 3, [lambda kb=kb: vtrans(kb)
                                         for kb in range(28, 32)])
            bg[4] += [lambda: shuffle_moves(0, 3)]

            chunks = [(0, 0), (0, 1), (0, 2), (0, 3),
                      (1, 0), (1, 1), (1, 2), (1, 3)]
            # proj of chunk c runs as fillers of chunk PROJ_AT[c]; proj(6)
            # runs after the loop (chunk 7 stays light so its exp stream
            # finishes early; proj(6) then overlaps chunk 7's scale chain)
            PROJ_AT = {0: 4, 1: 5, 2: 6, 3: 4, 4: 5, 5: 6}
            boxes = {}
            tail_prev = deque()
            box_prev = None
            for ci, (b, qb) in enumerate(chunks):
                fillers = deque(tail_prev)
                fillers.extend(bg[ci])
                for src, at in PROJ_AT.items():
                    if at == ci:
                        fillers.extend(proj_fillers(boxes[src], *chunks[src]))
                box_prev, tail_prev = emit_chunk(b, qb, fillers)
                boxes[ci] = box_prev
                while fillers:
                    fillers.popleft()()
                # deferred big loads: emitted on SP *after* this chunk's
                # DMAs so they queue behind the critical early transfers
                if ci == 0:
                    nc.sync.dma_start(out=woT_sb, in_=woT[:, :])
                    load_x(2, 0, 8)
                elif ci == 1:
                    load_x(3, 0, 8)
            while tail_prev:
                tail_prev.popleft()()
            # chunk-6 projection through the freed pav slots (2-way pipeline)
            pb6, pqb6 = chunks[6]
            for j in range(4):
                ob6 = opool.tile([128, 1024], F32, name="ob6")
                for eh in range(2):
                    po = psav.tile([128, 512], F32, tag=f"pav{eh}", name="po6")
                    nc.tensor.matmul(
                        po, boxes[6]["ctx"][:, j * 128:(j + 1) * 128],
                        woT_sb[:, eh * 512:(eh + 1) * 512],
                        start=True, stop=True)
                    nc.vector.tensor_copy(ob6[:, eh * 512:(eh + 1) * 512], po)
                nb = pqb6 * 4 + j
                nc.sync.dma_start(
                    out=out[pb6 * N + nb * 128: pb6 * N + (nb + 1) * 128, :],
                    in_=ob6)
            # final projection: scores psum is free now, use it for a
            # 2-deep pipeline instead of the single filler slot
            pb, pqb = chunks[-1]
            for j in range(4):
                po = pss.tile([128, 1024], F32, tag="pS", name="poF")
                nc.tensor.matmul(po[:, 0:512],
                                 box_prev["ctx"][:, j * 128:(j + 1) * 128],
                                 woT_sb[:, 0:512], start=True, stop=True)
                nc.tensor.matmul(po[:, 512:1024],
                                 box_prev["ctx"][:, j * 128:(j + 1) * 128],
                                 woT_sb[:, 512:1024], start=True, stop=True)
                ob = opool.tile([128, 1024], F32, name="obF")
                nc.vector.tensor_copy(ob, po)
                nb = pqb * 4 + j
                nc.sync.dma_start(
                    out=out[pb * N + nb * 128: pb * N + (nb + 1) * 128, :],
                    in_=ob)

    nc.compile()
    return nc


def _host_prep(x, W_qkv, b_qkv, W_out):
    x2 = np.ascontiguousarray(x.reshape(BN, D).T).astype(NP_BF)  # [D, BN]
    ident = np.eye(128, dtype=np.float32).astype(NP_BF)
    in_maps = []
    for c in range(NCORES):
        h0 = HPC * c
        rows = []
        for m in range(3):  # q, k, v
            for h in (h0, h0 + 1):
                lo = m * D + h * HD
                rows.extend(range(lo, lo + HD))
        rows = np.array(rows)
        wsel = W_qkv[rows, :]                              # [384, 1024]
        wqkvT = np.ascontiguousarray(wsel.T).astype(NP_BF)  # [1024, 384]
        bq = np.ascontiguousarray(b_qkv[rows].reshape(EPC, 1)).astype(np.float32)
        cols = np.arange(h0 * HD, h0 * HD + 2 * HD)        # ctx dims for this core
        woTc = np.ascontiguousarray(W_out[:, cols].T).astype(NP_BF)  # [128, 1024]
        in_maps.append({
            "xT": x2, "wqkvT": wqkvT, "bqkv": bq, "woT": woTc, "ident": ident,
        })
    return in_maps


def kernel(x, W_qkv, b_qkv, W_out, b_out, _trace=False):
    x = np.asarray(x, dtype=np.float32)
    W_qkv = np.asarray(W_qkv, dtype=np.float32)
    b_qkv = np.asarray(b_qkv, dtype=np.float32)
    W_out = np.asarray(W_out, dtype=np.float32)
    b_out = np.asarray(b_out, dtype=np.float32)

    if "nc" not in _cached:
        _cached["nc"] = build_nc()
    nc = _cached["nc"]

    in_maps = _host_prep(x, W_qkv, b_qkv, W_out)
    res = run_bass_kernel_spmd(nc, in_maps, list(range(NCORES)), trace=_trace)
    _cached["last_result"] = res

    total = np.zeros((BN, D), dtype=np.float64)
    for c in range(NCORES):
        total += res.results[c]["out"].astype(np.float64)
    total += b_out.astype(np.float64)
    return total.reshape(B, N, D).astype(np.float32)


if __name__ == "__main__":
    rng = np.random.default_rng(0)
    x = rng.standard_normal((B, N, D), dtype=np.float32)
    s = 1.0 / np.sqrt(D)
    W_qkv = rng.uniform(-s, s, (3 * D, D)).astype(np.float32)
    b_qkv = rng.uniform(-s, s, (3 * D,)).astype(np.float32)
    W_out = rng.uniform(-s, s, (D, D)).astype(np.float32)
    b_out = rng.uniform(-s, s, (D,)).astype(np.float32)
    got = kernel(x, W_qkv, b_qkv, W_out, b_out)
    print("kernel ran, out shape", got.shape)
